# revision 37
# baseline (speedup 1.0000x reference)
"""Trainium2 Bass kernel for nn_CustomConv2d: 3x3 conv, B=16, Cin=Cout=128, H=W=64.

Strategy (final, ~49.4-49.8us HW exec vs 52.8-54.7us fp32r baseline):
  - Data-parallel over batch: 8 NeuronCores x 2 images each; the (128,128,9)
    weight is replicated (host pre-transposes to [cin, tap, cout] so tap k is
    a contiguous [cin, cout] stationary-operand slice).
  - fp16 matmuls: 1 cycle/row like fp32r but the 2-byte LDWEIGHTS fully hides
    under the 512-row moving stream, so the measured matmul cadence is 216ns
    (the 213ns @2.4GHz streaming floor) vs 238ns for fp32r whose 186ns
    LDWEIGHTS only partially overlaps.  fp16's 10-bit mantissa keeps rel err
    at 2.7e-4 (gate 2e-2).  Input DMA bytes also halve.  PSUM stays fp32.
  - Per image the feature map lives in SBUF as a 66x66 zero-padded plane
    (host-prepadded => every DMA is contiguous per partition).
  - Conv = 9 accumulating PE matmuls per 8-row output block (contraction over
    Cin=128 on the partition dim; one fp32 PSUM bank per block).
  - DMA plan (from trace measurements): engines boot ~7.2us (fixed NEFF
    preamble); each HWDGE ring (sync=SP, scalar=Activation) starts moving
    data ~8.2/9.0us; each DMA costs ~0.5us fixed ring pacing; and a consumer
    sees a DMA's data only ~1.4-2.4us after its last descriptor retires
    (completion-semaphore latency).  Hence: few right-sized DMAs in strict
    consumption-deadline order, first chunk = exactly the 8 rows the first
    matmul window needs, w tap groups early on both rings, img1 last.
  - Dependency granularity is per-range (LDW waits only its tap's w DMA, the
    MM only its rows' chunk), so a single wt tile + single xin tile per image
    is optimal; splitting tiles was measured to slow the cruise.
  - PE warm-up: HAM boosts the PE clock 1.2->2.4GHz only after ~2.7us of
    sustained array activity and re-throttles after idle windows (a gap
    mid-stream costs the whole cruise: 216 -> 259ns cadence).  8 junk fp16
    matmuls on a memset tile bridge engine start (~8.0us) to data-ready
    (~11.6us) so conv runs boosted and gap-free.
  - Outputs: blocks ship in 2-block DMAs alternating rings; the final block
    is split by PARTITION into cout halves (64-descriptor DMAs, parallel
    vector+scalar copies, sync+scalar DMAs) to minimize the exit chain
    (copy -> 0.6us DMA config -> HWDGE -> ring -> completion sem -> drain).
"""

import numpy as np

import concourse.bass as bass  # noqa: F401  (registers bass types)
import concourse.tile as tile
import concourse.mybir as mybir
from concourse import bacc, bass_utils

F32 = mybir.dt.float32
F16 = mybir.dt.float16

B, CIN, COUT, KK, H, W = 16, 128, 128, 3, 64, 64
NCORES = 8
BPC = B // NCORES  # images per core
HW = H * W         # 4096
PW = W + 2         # padded row length (66)
PH = H + 2         # padded rows (66)
XLEN = PH * PW     # 4356
ROWBLK = 8         # output rows per PSUM block (8*64=512 = one fp32 PSUM bank)
NBLK = H // ROWBLK # 8 blocks per image

WARMN = 8          # warmup matmuls (bridge engine-start -> first data ready)
TRACE = False      # set True to capture an NTFF profile (fills LAST_EXEC_NS)
LAST_EXEC_NS = None

_CACHE = {}

# img0 x chunks (padded-row ranges), consumption-ordered; block yb needs rows
# [8yb, 8yb+10).  The first conv matmul (tap dy=0) needs only rows 0-7, and
# consumer visibility = chunk ring-completion + ~2.4us semaphore latency, so
# the first chunk is exactly rows 0-7.  img1 in 3 coarser chunks (~28us+).
CHUNKS0 = [(0, 8), (8, 18), (18, 34), (34, 50), (50, PH)]
CHUNKS1 = [(0, 22), (22, 44), (44, PH)]


def _build():
    nc = bacc.Bacc("TRN2", target_bir_lowering=False, debug=False, num_devices=NCORES)
    x_d = nc.dram_tensor("x", [BPC, CIN, XLEN], F16, kind="ExternalInput").ap()
    w_d = nc.dram_tensor("w", [CIN, KK * KK * COUT], F16, kind="ExternalInput").ap()
    o_d = nc.dram_tensor("o", [BPC, COUT, HW], F16, kind="ExternalOutput").ap()

    with tile.TileContext(nc) as tc:
        with (
            tc.tile_pool(name="wt", bufs=1) as wtp,
            tc.tile_pool(name="xin", bufs=2) as xp,
            tc.tile_pool(name="ps", bufs=6, space="PSUM") as pp,
            tc.tile_pool(name="ot", bufs=4) as op,
            tc.tile_pool(name="warm", bufs=1) as wmp,
            tc.tile_pool(name="warmps", bufs=1, space="PSUM") as wpp,
        ):
            # --- warmup: keep the PE busy from engine start until data lands.
            wz = wmp.tile([CIN, 4 * COUT], F16)
            nc.vector.memset(wz[:], 0.0)
            wps = wpp.tile([COUT, 4 * COUT], F32)
            for _ in range(WARMN):
                nc.tensor.matmul(wps[:], wz[:, :COUT], wz[:], start=True, stop=True)

            # --- input DMAs.  The two HWDGE engines (sync, scalar) each own a
            # ring; per-DMA ring pacing (~0.5us fixed + bytes) is the early
            # bottleneck, so chunks are as coarse as their deadlines allow.
            wt = wtp.tile([CIN, KK * KK * COUT], F16)
            xins = []
            for lb in range(BPC):
                xin = xp.tile([CIN, XLEN], F16, tag="xin")
                xins.append(xin)

            def xc(lb, r0, r1):
                return (xins[lb][:, PW * r0 : PW * r1], x_d[lb][:, PW * r0 : PW * r1])

            wg = [
                (wt[:, g * 3 * COUT : (g + 1) * 3 * COUT],
                 w_d[:, g * 3 * COUT : (g + 1) * 3 * COUT])
                for g in range(3)
            ]
            ring = [nc.sync, nc.scalar]
            c = [xc(0, r0, r1) for r0, r1 in CHUNKS0]
            i1 = [xc(1, r0, r1) for r0, r1 in CHUNKS1]
            # sync's ring starts ~0.8us earlier; it carries w taps 0-2 and the
            # first-matmul rows.  Everything ordered by consumption deadline;
            # each DMA costs ~0.5us of fixed ring pacing, so chunks are merged
            # up to the granularity their deadline allows.
            sync_plan = [wg[0], c[0], c[1], c[3], c[4], i1[1]]
            scalar_plan = [wg[1], wg[2], c[2], i1[0], i1[2]]
            for eng, plan in ((nc.sync, sync_plan), (nc.scalar, scalar_plan)):
                for dst, src in plan:
                    eng.dma_start(dst, src)

            # --- conv: 9 accumulating matmuls per 8-row block ---
            nout = 0
            for lb in range(BPC):
                xrf = xins[lb][:].rearrange("p (r c) -> p r c", c=PW)  # [128,66,66]
                for yb in range(NBLK):
                    y0 = yb * ROWBLK
                    ps = pp.tile([COUT, ROWBLK * W], F32)
                    first = True
                    for dy in range(KK):
                        for dx in range(KK):
                            t = dy * KK + dx
                            nc.tensor.matmul(
                                ps[:],
                                wt[:, t * COUT : (t + 1) * COUT],
                                xrf[:, y0 + dy : y0 + dy + ROWBLK, dx : dx + W],
                                start=first,
                                stop=(dy == KK - 1 and dx == KK - 1),
                            )
                            first = False
                    if lb == BPC - 1 and yb == NBLK - 1:
                        ot = op.tile([COUT, ROWBLK * W], F16, tag="otf")
                        # final block split by PARTITION (cout halves): copies
                        # run on vector+scalar in parallel and each output DMA
                        # only needs 64 descriptors, halving the exit chain.
                        ph = COUT // 2
                        for h_, deng in ((0, nc.sync), (1, nc.scalar)):
                            sl = slice(h_ * ph, (h_ + 1) * ph)
                            if h_ == 0:
                                nc.vector.tensor_copy(ot[sl, :], ps[sl, :])
                            else:
                                nc.scalar.copy(ot[sl, :], ps[sl, :])
                            deng.dma_start(
                                o_d[lb][sl, W * y0 : W * y0 + ROWBLK * W],
                                ot[sl, :],
                            )
                    elif lb == BPC - 1 and yb == NBLK - 2:
                        # block before the final one ships alone (its pair
                        # partner takes the exit-critical path)
                        ot = op.tile([COUT, ROWBLK * W], F16, tag="ot1")
                        nc.vector.tensor_copy(ot[:], ps[:])
                        ring[nout % 2].dma_start(
                            o_d[lb][:, W * y0 : W * y0 + ROWBLK * W], ot[:]
                        )
                        nout += 1
                    else:
                        # stage two blocks per SBUF tile; ship them as one
                        # 2-block DMA (fewer DMAs = less ring pacing overhead
                        # ahead of the exit-critical final DMA)
                        if yb % 2 == 0:
                            otp = op.tile([COUT, 2 * ROWBLK * W], F16, tag="otp")
                        nc.vector.tensor_copy(
                            otp[:, (yb % 2) * ROWBLK * W : (yb % 2 + 1) * ROWBLK * W],
                            ps[:],
                        )
                        if yb % 2 == 1:
                            ring[nout % 2].dma_start(
                                o_d[lb][:, W * (y0 - ROWBLK) : W * y0 + ROWBLK * W],
                                otp[:],
                            )
                            nout += 1
    nc.compile()
    return nc


def _get_nc():
    key = ("nc_v13", WARMN)
    if key not in _CACHE:
        _CACHE[key] = _build()
    return _CACHE[key]


def kernel(x, weights):
    """x: [16,128,64,64] f32; weights: [128,128,9] f32 -> [2048,64,64] f32."""
    global LAST_EXEC_NS
    x = np.asarray(x, dtype=np.float32)
    w = np.asarray(weights, dtype=np.float32)
    # [cout, cin, k] -> [cin, k, cout] so tap k is a contiguous lhsT slice
    wT = np.ascontiguousarray(w.transpose(1, 2, 0)).reshape(CIN, KK * KK * COUT)
    wT = wT.astype(np.float16)
    xpad = np.zeros((B, CIN, PH, PW), np.float16)
    xpad[:, :, 1 : H + 1, 1 : W + 1] = x.astype(np.float16)
    xpad = xpad.reshape(B, CIN, XLEN)

    nc = _get_nc()
    xr = xpad.reshape(NCORES, BPC, CIN, XLEN)
    in_maps = [{"x": np.ascontiguousarray(xr[c]), "w": wT} for c in range(NCORES)]

    res = bass_utils.run_bass_kernel_spmd(
        nc, in_maps, core_ids=list(range(NCORES)), trace=TRACE
    )
    LAST_EXEC_NS = res.exec_time_ns

    arr = np.stack([res.results[c]["o"] for c in range(NCORES)]).astype(np.float32)
    # out[cout*B + b] = conv[b, cout], with b = core*BPC + lb
    arr = arr.transpose(2, 0, 1, 3).reshape(COUT, B, H, W)
    return np.ascontiguousarray(arr.reshape(COUT * B, H, W))


# revision 38
# speedup vs baseline: 1.0124x; 1.0124x over previous
"""Trainium2 Bass kernel for nn_CustomConv2d: 3x3 conv, B=16, Cin=Cout=128, H=W=64.

Strategy (final, ~49.4-49.8us HW exec vs 52.8-54.7us fp32r baseline):
  - Data-parallel over batch: 8 NeuronCores x 2 images each; the (128,128,9)
    weight is replicated (host pre-transposes to [cin, tap, cout] so tap k is
    a contiguous [cin, cout] stationary-operand slice).
  - fp16 matmuls: 1 cycle/row like fp32r but the 2-byte LDWEIGHTS fully hides
    under the 512-row moving stream, so the measured matmul cadence is 216ns
    (the 213ns @2.4GHz streaming floor) vs 238ns for fp32r whose 186ns
    LDWEIGHTS only partially overlaps.  fp16's 10-bit mantissa keeps rel err
    at 2.7e-4 (gate 2e-2).  Input DMA bytes also halve.  PSUM stays fp32.
  - Per image the feature map lives in SBUF as a 66x66 zero-padded plane
    (host-prepadded => every DMA is contiguous per partition).
  - Conv = 9 accumulating PE matmuls per 8-row output block (contraction over
    Cin=128 on the partition dim; one fp32 PSUM bank per block).
  - DMA plan (from trace measurements): engines boot ~7.2us (fixed NEFF
    preamble); each HWDGE ring (sync=SP, scalar=Activation) starts moving
    data ~8.2/9.0us; each DMA costs ~0.5us fixed ring pacing; and a consumer
    sees a DMA's data only ~1.4-2.4us after its last descriptor retires
    (completion-semaphore latency).  Hence: few right-sized DMAs in strict
    consumption-deadline order, first chunk = exactly the 8 rows the first
    matmul window needs, w tap groups early on both rings, img1 last.
  - Dependency granularity is per-range (LDW waits only its tap's w DMA, the
    MM only its rows' chunk), so a single wt tile + single xin tile per image
    is optimal; splitting tiles was measured to slow the cruise.
  - PE warm-up: HAM boosts the PE clock 1.2->2.4GHz only after ~2.7us of
    sustained array activity and re-throttles after idle windows (a gap
    mid-stream costs the whole cruise: 216 -> 259ns cadence).  8 junk fp16
    matmuls on a memset tile bridge engine start (~8.0us) to data-ready
    (~11.6us) so conv runs boosted and gap-free.
  - Outputs: blocks ship in 2-block DMAs alternating rings; the final block
    is split by PARTITION into cout halves (64-descriptor DMAs, parallel
    vector+scalar copies, sync+scalar DMAs) to minimize the exit chain
    (copy -> 0.6us DMA config -> HWDGE -> ring -> completion sem -> drain).
"""

import numpy as np

import concourse.bass as bass  # noqa: F401  (registers bass types)
import concourse.tile as tile
import concourse.mybir as mybir
from concourse import bacc, bass_utils

F32 = mybir.dt.float32
F16 = mybir.dt.float16

B, CIN, COUT, KK, H, W = 16, 128, 128, 3, 64, 64
NCORES = 8
BPC = B // NCORES  # images per core
HW = H * W         # 4096
PW = W + 2         # padded row length (66)
PH = H + 2         # padded rows (66)
XLEN = PH * PW     # 4356
ROWBLK = 8         # output rows per PSUM block (8*64=512 = one fp32 PSUM bank)
NBLK = H // ROWBLK # 8 blocks per image

WARMN = 8          # warmup matmuls (bridge engine-start -> first data ready)
TRACE = False      # set True to capture an NTFF profile (fills LAST_EXEC_NS)
LAST_EXEC_NS = None

_CACHE = {}

# img0 x chunks (padded-row ranges), consumption-ordered; block yb needs rows
# [8yb, 8yb+10).  The first conv matmul (tap dy=0) needs only rows 0-7, and
# consumer visibility = chunk ring-completion + ~2.4us semaphore latency, so
# the first chunk is exactly rows 0-7.  img1 in 3 coarser chunks (~28us+).
CHUNKS0 = [(0, 8), (8, 18), (18, 34), (34, 50), (50, PH)]
CHUNKS1 = [(0, 22), (22, 44), (44, PH)]


def _build():
    nc = bacc.Bacc("TRN2", target_bir_lowering=False, debug=False, num_devices=NCORES)
    x_d = nc.dram_tensor("x", [BPC, CIN, XLEN], F16, kind="ExternalInput").ap()
    w_d = nc.dram_tensor("w", [CIN, KK * KK * COUT], F16, kind="ExternalInput").ap()
    o_d = nc.dram_tensor("o", [BPC, COUT, HW], F32, kind="ExternalOutput").ap()

    with tile.TileContext(nc) as tc:
        with (
            tc.tile_pool(name="wt", bufs=1) as wtp,
            tc.tile_pool(name="xin", bufs=2) as xp,
            tc.tile_pool(name="ps", bufs=6, space="PSUM") as pp,
            tc.tile_pool(name="ot", bufs=4) as op,
            tc.tile_pool(name="warm", bufs=1) as wmp,
            tc.tile_pool(name="warmps", bufs=1, space="PSUM") as wpp,
        ):
            # --- warmup: keep the PE busy from engine start until data lands.
            wz = wmp.tile([CIN, 4 * COUT], F16)
            nc.vector.memset(wz[:], 0.0)
            wps = wpp.tile([COUT, 4 * COUT], F32)
            for _ in range(WARMN):
                nc.tensor.matmul(wps[:], wz[:, :COUT], wz[:], start=True, stop=True)

            # --- input DMAs.  The two HWDGE engines (sync, scalar) each own a
            # ring; per-DMA ring pacing (~0.5us fixed + bytes) is the early
            # bottleneck, so chunks are as coarse as their deadlines allow.
            wt = wtp.tile([CIN, KK * KK * COUT], F16)
            xins = []
            for lb in range(BPC):
                xin = xp.tile([CIN, XLEN], F16, tag="xin")
                xins.append(xin)

            def xc(lb, r0, r1):
                return (xins[lb][:, PW * r0 : PW * r1], x_d[lb][:, PW * r0 : PW * r1])

            wg = [
                (wt[:, g * 3 * COUT : (g + 1) * 3 * COUT],
                 w_d[:, g * 3 * COUT : (g + 1) * 3 * COUT])
                for g in range(3)
            ]
            ring = [nc.sync, nc.scalar]
            c = [xc(0, r0, r1) for r0, r1 in CHUNKS0]
            i1 = [xc(1, r0, r1) for r0, r1 in CHUNKS1]
            # sync's ring starts ~0.8us earlier; it carries w taps 0-2 and the
            # first-matmul rows.  Everything ordered by consumption deadline;
            # each DMA costs ~0.5us of fixed ring pacing, so chunks are merged
            # up to the granularity their deadline allows.
            sync_plan = [wg[0], c[0], c[1], c[3], c[4], i1[1]]
            scalar_plan = [wg[1], wg[2], c[2], i1[0], i1[2]]
            for eng, plan in ((nc.sync, sync_plan), (nc.scalar, scalar_plan)):
                for dst, src in plan:
                    eng.dma_start(dst, src)

            # --- conv: 9 accumulating matmuls per 8-row block ---
            nout = 0
            for lb in range(BPC):
                xrf = xins[lb][:].rearrange("p (r c) -> p r c", c=PW)  # [128,66,66]
                for yb in range(NBLK):
                    y0 = yb * ROWBLK
                    ps = pp.tile([COUT, ROWBLK * W], F32)
                    first = True
                    for dy in range(KK):
                        for dx in range(KK):
                            t = dy * KK + dx
                            nc.tensor.matmul(
                                ps[:],
                                wt[:, t * COUT : (t + 1) * COUT],
                                xrf[:, y0 + dy : y0 + dy + ROWBLK, dx : dx + W],
                                start=first,
                                stop=(dy == KK - 1 and dx == KK - 1),
                            )
                            first = False
                    if lb == BPC - 1 and yb == NBLK - 1:
                        ot = op.tile([COUT, ROWBLK * W], F32, tag="otf")
                        # final block split by PARTITION (cout halves): copies
                        # run on vector+scalar in parallel and each output DMA
                        # only needs 64 descriptors, halving the exit chain.
                        ph = COUT // 2
                        for h_, deng in ((0, nc.sync), (1, nc.scalar)):
                            sl = slice(h_ * ph, (h_ + 1) * ph)
                            if h_ == 0:
                                nc.vector.tensor_copy(ot[sl, :], ps[sl, :])
                            else:
                                nc.scalar.copy(ot[sl, :], ps[sl, :])
                            deng.dma_start(
                                o_d[lb][sl, W * y0 : W * y0 + ROWBLK * W],
                                ot[sl, :],
                            )
                    elif lb == BPC - 1 and yb == NBLK - 2:
                        # block before the final one ships alone (its pair
                        # partner takes the exit-critical path)
                        ot = op.tile([COUT, ROWBLK * W], F32, tag="ot1")
                        nc.vector.tensor_copy(ot[:], ps[:])
                        ring[nout % 2].dma_start(
                            o_d[lb][:, W * y0 : W * y0 + ROWBLK * W], ot[:]
                        )
                        nout += 1
                    else:
                        # stage two blocks per SBUF tile; ship them as one
                        # 2-block DMA (fewer DMAs = less ring pacing overhead
                        # ahead of the exit-critical final DMA)
                        if yb % 2 == 0:
                            otp = op.tile([COUT, 2 * ROWBLK * W], F32, tag="otp")
                        nc.vector.tensor_copy(
                            otp[:, (yb % 2) * ROWBLK * W : (yb % 2 + 1) * ROWBLK * W],
                            ps[:],
                        )
                        if yb % 2 == 1:
                            ring[nout % 2].dma_start(
                                o_d[lb][:, W * (y0 - ROWBLK) : W * y0 + ROWBLK * W],
                                otp[:],
                            )
                            nout += 1
    nc.compile()
    return nc


def _get_nc():
    key = ("nc_v12f", WARMN)
    if key not in _CACHE:
        _CACHE[key] = _build()
    return _CACHE[key]


def kernel(x, weights):
    """x: [16,128,64,64] f32; weights: [128,128,9] f32 -> [2048,64,64] f32."""
    global LAST_EXEC_NS
    x = np.asarray(x, dtype=np.float32)
    w = np.asarray(weights, dtype=np.float32)
    # [cout, cin, k] -> [cin, k, cout] so tap k is a contiguous lhsT slice
    wT = np.ascontiguousarray(w.transpose(1, 2, 0)).reshape(CIN, KK * KK * COUT)
    wT = wT.astype(np.float16)
    xpad = np.zeros((B, CIN, PH, PW), np.float16)
    xpad[:, :, 1 : H + 1, 1 : W + 1] = x.astype(np.float16)
    xpad = xpad.reshape(B, CIN, XLEN)

    nc = _get_nc()
    xr = xpad.reshape(NCORES, BPC, CIN, XLEN)
    in_maps = [{"x": np.ascontiguousarray(xr[c]), "w": wT} for c in range(NCORES)]

    res = bass_utils.run_bass_kernel_spmd(
        nc, in_maps, core_ids=list(range(NCORES)), trace=TRACE
    )
    LAST_EXEC_NS = res.exec_time_ns

    arr = np.stack([res.results[c]["o"] for c in range(NCORES)]).astype(np.float32)
    # out[cout*B + b] = conv[b, cout], with b = core*BPC + lb
    arr = arr.transpose(2, 0, 1, 3).reshape(COUT, B, H, W)
    return np.ascontiguousarray(arr.reshape(COUT * B, H, W))


# revision 39
# speedup vs baseline: 1.0273x; 1.0148x over previous
"""Trainium2 Bass kernel for nn_CustomConv2d: 3x3 conv, B=16, Cin=Cout=128, H=W=64.

Strategy (final, ~49.4-49.8us HW exec vs 52.8-54.7us fp32r baseline):
  - Data-parallel over batch: 8 NeuronCores x 2 images each; the (128,128,9)
    weight is replicated (host pre-transposes to [cin, tap, cout] so tap k is
    a contiguous [cin, cout] stationary-operand slice).
  - fp16 matmuls: 1 cycle/row like fp32r but the 2-byte LDWEIGHTS fully hides
    under the 512-row moving stream, so the measured matmul cadence is 216ns
    (the 213ns @2.4GHz streaming floor) vs 238ns for fp32r whose 186ns
    LDWEIGHTS only partially overlaps.  fp16's 10-bit mantissa keeps rel err
    at 2.7e-4 (gate 2e-2).  Input DMA bytes also halve.  PSUM stays fp32.
  - Per image the feature map lives in SBUF as a 66x66 zero-padded plane
    (host-prepadded => every DMA is contiguous per partition).
  - Conv = 9 accumulating PE matmuls per 8-row output block (contraction over
    Cin=128 on the partition dim; one fp32 PSUM bank per block).
  - DMA plan (from trace measurements): engines boot ~7.2us (fixed NEFF
    preamble); each HWDGE ring (sync=SP, scalar=Activation) starts moving
    data ~8.2/9.0us; each DMA costs ~0.5us fixed ring pacing; and a consumer
    sees a DMA's data only ~1.4-2.4us after its last descriptor retires
    (completion-semaphore latency).  Hence: few right-sized DMAs in strict
    consumption-deadline order, first chunk = exactly the 8 rows the first
    matmul window needs, w tap groups early on both rings, img1 last.
  - Dependency granularity is per-range (LDW waits only its tap's w DMA, the
    MM only its rows' chunk), so a single wt tile + single xin tile per image
    is optimal; splitting tiles was measured to slow the cruise.
  - PE warm-up: HAM boosts the PE clock 1.2->2.4GHz only after ~2.7us of
    sustained array activity and re-throttles after idle windows (a gap
    mid-stream costs the whole cruise: 216 -> 259ns cadence).  8 junk fp16
    matmuls on a memset tile bridge engine start (~8.0us) to data-ready
    (~11.6us) so conv runs boosted and gap-free.
  - Outputs: blocks ship in 2-block DMAs alternating rings; the final block
    is split by PARTITION into cout halves (64-descriptor DMAs, parallel
    vector+scalar copies, sync+scalar DMAs) to minimize the exit chain
    (copy -> 0.6us DMA config -> HWDGE -> ring -> completion sem -> drain).
"""

import numpy as np

import concourse.bass as bass  # noqa: F401  (registers bass types)
import concourse.tile as tile
import concourse.mybir as mybir
from concourse import bacc, bass_utils

F32 = mybir.dt.float32
F16 = mybir.dt.float16

B, CIN, COUT, KK, H, W = 16, 128, 128, 3, 64, 64
NCORES = 8
BPC = B // NCORES  # images per core
HW = H * W         # 4096
PW = W + 2         # padded row length (66)
PH = H + 2         # padded rows (66)
XLEN = PH * PW     # 4356
ROWBLK = 8         # output rows per PSUM block (8*64=512 = one fp32 PSUM bank)
NBLK = H // ROWBLK # 8 blocks per image

WARMN = 8          # warmup matmuls (bridge engine-start -> first data ready)
TRACE = False      # set True to capture an NTFF profile (fills LAST_EXEC_NS)
LAST_EXEC_NS = None

_CACHE = {}

# img0 x chunks (padded-row ranges), consumption-ordered; block yb needs rows
# [8yb, 8yb+10).  The first conv matmul (tap dy=0) needs only rows 0-7, and
# consumer visibility = chunk ring-completion + ~2.4us semaphore latency, so
# the first chunk is exactly rows 0-7.  img1 in 3 coarser chunks (~28us+).
CHUNKS0 = [(0, 8), (8, 18), (18, 34), (34, 50), (50, PH)]
CHUNKS1 = [(0, 22), (22, 44), (44, PH)]


def _build():
    nc = bacc.Bacc("TRN2", target_bir_lowering=False, debug=False, num_devices=NCORES)
    WPFX = KK * KK * COUT  # 1152-col w prefix ahead of each image plane
    x_d = nc.dram_tensor("x", [BPC, CIN, WPFX + XLEN], F16, kind="ExternalInput").ap()
    o_d = nc.dram_tensor("o", [BPC, COUT, HW], F32, kind="ExternalOutput").ap()

    with tile.TileContext(nc) as tc:
        with (
            tc.tile_pool(name="wt", bufs=1) as wtp,
            tc.tile_pool(name="xin", bufs=2) as xp,
            tc.tile_pool(name="ps", bufs=6, space="PSUM") as pp,
            tc.tile_pool(name="ot", bufs=4) as op,
            tc.tile_pool(name="warm", bufs=1) as wmp,
            tc.tile_pool(name="warmps", bufs=1, space="PSUM") as wpp,
        ):
            # --- warmup: keep the PE busy from engine start until data lands.
            wz = wmp.tile([CIN, 4 * COUT], F16)
            nc.vector.memset(wz[:], 0.0)
            wps = wpp.tile([COUT, 4 * COUT], F32)
            for _ in range(WARMN):
                nc.tensor.matmul(wps[:], wz[:, :COUT], wz[:], start=True, stop=True)

            # --- input DMAs.  The two HWDGE engines (sync, scalar) each own a
            # ring; per-DMA ring pacing (~0.5us fixed + bytes) is the early
            # bottleneck, so chunks are as coarse as their deadlines allow.
            # The w taps ride as a 1152-col prefix of img0's plane, so ONE
            # leading DMA delivers all 9 taps plus the first matmul's 8 rows.
            xins = []
            for lb in range(BPC):
                xin = xp.tile([CIN, WPFX + XLEN], F16, tag="xin")
                xins.append(xin)
            wt = xins[0]

            def xc(lb, r0, r1):
                return (
                    xins[lb][:, WPFX + PW * r0 : WPFX + PW * r1],
                    x_d[lb][:, WPFX + PW * r0 : WPFX + PW * r1],
                )

            ring = [nc.sync, nc.scalar]
            wc0 = (xins[0][:, : WPFX + PW * CHUNKS0[0][1]],
                   x_d[0][:, : WPFX + PW * CHUNKS0[0][1]])
            c = [None] + [xc(0, r0, r1) for r0, r1 in CHUNKS0[1:]]
            i1 = [xc(1, r0, r1) for r0, r1 in CHUNKS1]
            sync_plan = [wc0, c[1], c[3], i1[1]]
            scalar_plan = [c[2], c[4], i1[0], i1[2]]
            for eng, plan in ((nc.sync, sync_plan), (nc.scalar, scalar_plan)):
                for dst, src in plan:
                    eng.dma_start(dst, src)

            # --- conv: 9 accumulating matmuls per 8-row block ---
            nout = 0
            for lb in range(BPC):
                xrf = xins[lb][:, WPFX:].rearrange("p (r c) -> p r c", c=PW)
                for yb in range(NBLK):
                    y0 = yb * ROWBLK
                    ps = pp.tile([COUT, ROWBLK * W], F32)
                    first = True
                    for dy in range(KK):
                        for dx in range(KK):
                            t = dy * KK + dx
                            nc.tensor.matmul(
                                ps[:],
                                wt[:, t * COUT : (t + 1) * COUT],
                                xrf[:, y0 + dy : y0 + dy + ROWBLK, dx : dx + W],
                                start=first,
                                stop=(dy == KK - 1 and dx == KK - 1),
                            )
                            first = False
                    if lb == BPC - 1 and yb == NBLK - 1:
                        ot = op.tile([COUT, ROWBLK * W], F32, tag="otf")
                        # final block split by PARTITION (cout halves): copies
                        # run on vector+scalar in parallel and each output DMA
                        # only needs 64 descriptors, halving the exit chain.
                        ph = COUT // 2
                        for h_, deng in ((0, nc.sync), (1, nc.scalar)):
                            sl = slice(h_ * ph, (h_ + 1) * ph)
                            if h_ == 0:
                                nc.vector.tensor_copy(ot[sl, :], ps[sl, :])
                            else:
                                nc.scalar.copy(ot[sl, :], ps[sl, :])
                            deng.dma_start(
                                o_d[lb][sl, W * y0 : W * y0 + ROWBLK * W],
                                ot[sl, :],
                            )
                    elif lb == BPC - 1 and yb == NBLK - 2:
                        # block before the final one ships alone (its pair
                        # partner takes the exit-critical path)
                        ot = op.tile([COUT, ROWBLK * W], F32, tag="ot1")
                        nc.vector.tensor_copy(ot[:], ps[:])
                        ring[nout % 2].dma_start(
                            o_d[lb][:, W * y0 : W * y0 + ROWBLK * W], ot[:]
                        )
                        nout += 1
                    else:
                        # stage two blocks per SBUF tile; ship them as one
                        # 2-block DMA (fewer DMAs = less ring pacing overhead
                        # ahead of the exit-critical final DMA)
                        if yb % 2 == 0:
                            otp = op.tile([COUT, 2 * ROWBLK * W], F32, tag="otp")
                        nc.vector.tensor_copy(
                            otp[:, (yb % 2) * ROWBLK * W : (yb % 2 + 1) * ROWBLK * W],
                            ps[:],
                        )
                        if yb % 2 == 1:
                            ring[nout % 2].dma_start(
                                o_d[lb][:, W * (y0 - ROWBLK) : W * y0 + ROWBLK * W],
                                otp[:],
                            )
                            nout += 1
    nc.compile()
    return nc


def _get_nc():
    key = ("nc_v14", WARMN)
    if key not in _CACHE:
        _CACHE[key] = _build()
    return _CACHE[key]


def kernel(x, weights):
    """x: [16,128,64,64] f32; weights: [128,128,9] f32 -> [2048,64,64] f32."""
    global LAST_EXEC_NS
    x = np.asarray(x, dtype=np.float32)
    w = np.asarray(weights, dtype=np.float32)
    # [cout, cin, k] -> [cin, k, cout] so tap k is a contiguous lhsT slice
    WPFX = KK * KK * COUT
    wT = np.ascontiguousarray(w.transpose(1, 2, 0)).reshape(CIN, WPFX)
    wT = wT.astype(np.float16)
    xpad = np.zeros((B, CIN, WPFX + XLEN), np.float16)
    xpad[:, :, :WPFX] = wT
    xpad[:, :, WPFX:] = np.pad(
        x.astype(np.float16), ((0, 0), (0, 0), (1, 1), (1, 1))
    ).reshape(B, CIN, XLEN)

    nc = _get_nc()
    xr = xpad.reshape(NCORES, BPC, CIN, WPFX + XLEN)
    in_maps = [{"x": np.ascontiguousarray(xr[c])} for c in range(NCORES)]

    res = bass_utils.run_bass_kernel_spmd(
        nc, in_maps, core_ids=list(range(NCORES)), trace=TRACE
    )
    LAST_EXEC_NS = res.exec_time_ns

    arr = np.stack([res.results[c]["o"] for c in range(NCORES)]).astype(np.float32)
    # out[cout*B + b] = conv[b, cout], with b = core*BPC + lb
    arr = arr.transpose(2, 0, 1, 3).reshape(COUT, B, H, W)
    return np.ascontiguousarray(arr.reshape(COUT * B, H, W))


# revision 40
# speedup vs baseline: 1.0336x; 1.0061x over previous
"""Trainium2 Bass kernel for nn_CustomConv2d: 3x3 conv, B=16, Cin=Cout=128, H=W=64.

Strategy (final, ~49.4-49.8us HW exec vs 52.8-54.7us fp32r baseline):
  - Data-parallel over batch: 8 NeuronCores x 2 images each; the (128,128,9)
    weight is replicated (host pre-transposes to [cin, tap, cout] so tap k is
    a contiguous [cin, cout] stationary-operand slice).
  - fp16 matmuls: 1 cycle/row like fp32r but the 2-byte LDWEIGHTS fully hides
    under the 512-row moving stream, so the measured matmul cadence is 216ns
    (the 213ns @2.4GHz streaming floor) vs 238ns for fp32r whose 186ns
    LDWEIGHTS only partially overlaps.  fp16's 10-bit mantissa keeps rel err
    at 2.7e-4 (gate 2e-2).  Input DMA bytes also halve.  PSUM stays fp32.
  - Per image the feature map lives in SBUF as a 66x66 zero-padded plane
    (host-prepadded => every DMA is contiguous per partition).
  - Conv = 9 accumulating PE matmuls per 8-row output block (contraction over
    Cin=128 on the partition dim; one fp32 PSUM bank per block).
  - DMA plan (from trace measurements): engines boot ~7.2us (fixed NEFF
    preamble); each HWDGE ring (sync=SP, scalar=Activation) starts moving
    data ~8.2/9.0us; each DMA costs ~0.5us fixed ring pacing; and a consumer
    sees a DMA's data only ~1.4-2.4us after its last descriptor retires
    (completion-semaphore latency).  Hence: few right-sized DMAs in strict
    consumption-deadline order, first chunk = exactly the 8 rows the first
    matmul window needs, w tap groups early on both rings, img1 last.
  - Dependency granularity is per-range (LDW waits only its tap's w DMA, the
    MM only its rows' chunk), so a single wt tile + single xin tile per image
    is optimal; splitting tiles was measured to slow the cruise.
  - PE warm-up: HAM boosts the PE clock 1.2->2.4GHz only after ~2.7us of
    sustained array activity and re-throttles after idle windows (a gap
    mid-stream costs the whole cruise: 216 -> 259ns cadence).  8 junk fp16
    matmuls on a memset tile bridge engine start (~8.0us) to data-ready
    (~11.6us) so conv runs boosted and gap-free.
  - Outputs: blocks ship in 2-block DMAs alternating rings; the final block
    is split by PARTITION into cout halves (64-descriptor DMAs, parallel
    vector+scalar copies, sync+scalar DMAs) to minimize the exit chain
    (copy -> 0.6us DMA config -> HWDGE -> ring -> completion sem -> drain).
"""

import numpy as np

import concourse.bass as bass  # noqa: F401  (registers bass types)
import concourse.tile as tile
import concourse.mybir as mybir
from concourse import bacc, bass_utils

F32 = mybir.dt.float32
F16 = mybir.dt.float16

B, CIN, COUT, KK, H, W = 16, 128, 128, 3, 64, 64
NCORES = 8
BPC = B // NCORES  # images per core
HW = H * W         # 4096
PW = W + 2         # padded row length (66)
PH = H + 2         # padded rows (66)
XLEN = PH * PW     # 4356
ROWBLK = 8         # output rows per PSUM block (8*64=512 = one fp32 PSUM bank)
NBLK = H // ROWBLK # 8 blocks per image

WARMN = 7          # warmup matmuls (bridge engine-start -> first data ready)
TRACE = False      # set True to capture an NTFF profile (fills LAST_EXEC_NS)
LAST_EXEC_NS = None

_CACHE = {}

# img0 x chunks (padded-row ranges), consumption-ordered; block yb needs rows
# [8yb, 8yb+10).  The first conv matmul (tap dy=0) needs only rows 0-7, and
# consumer visibility = chunk ring-completion + ~2.4us semaphore latency, so
# the first chunk is exactly rows 0-7.  img1 in 3 coarser chunks (~28us+).
CHUNKS0 = [(0, 8), (8, 18), (18, 34), (34, 50), (50, PH)]
CHUNKS1 = [(0, 22), (22, 44), (44, PH)]


def _build():
    nc = bacc.Bacc("TRN2", target_bir_lowering=False, debug=False, num_devices=NCORES)
    WPFX = 3 * COUT        # taps 0-2 prefix ahead of each image plane
    WSFX = 6 * COUT        # taps 3-8 suffix after the plane
    XW = WPFX + XLEN + WSFX
    x_d = nc.dram_tensor("x", [BPC, CIN, XW], F16, kind="ExternalInput").ap()
    o_d = nc.dram_tensor("o", [BPC, COUT, HW], F32, kind="ExternalOutput").ap()

    with tile.TileContext(nc) as tc:
        with (
            tc.tile_pool(name="wt", bufs=1) as wtp,
            tc.tile_pool(name="xin", bufs=2) as xp,
            tc.tile_pool(name="ps", bufs=6, space="PSUM") as pp,
            tc.tile_pool(name="ot", bufs=4) as op,
            tc.tile_pool(name="warm", bufs=1) as wmp,
            tc.tile_pool(name="warmps", bufs=1, space="PSUM") as wpp,
        ):
            # --- warmup: keep the PE busy from engine start until data lands.
            wz = wmp.tile([CIN, 4 * COUT], F16)
            nc.vector.memset(wz[:], 0.0)
            wps = wpp.tile([COUT, 4 * COUT], F32)
            for _ in range(WARMN):
                nc.tensor.matmul(wps[:], wz[:, :COUT], wz[:], start=True, stop=True)

            # --- input DMAs.  The two HWDGE engines (sync, scalar) each own a
            # ring; per-DMA ring pacing (~0.5us fixed + bytes) is the early
            # bottleneck, so chunks are as coarse as their deadlines allow.
            # The w taps ride as a 1152-col prefix of img0's plane, so ONE
            # leading DMA delivers all 9 taps plus the first matmul's 8 rows.
            xins = []
            for lb in range(BPC):
                xin = xp.tile([CIN, XW], F16, tag="xin")
                xins.append(xin)

            def lhs(t):  # stationary slice for tap t (prefix taps 0-2, suffix 3-8)
                if t < 3:
                    return xins[0][:, t * COUT : (t + 1) * COUT]
                o = WPFX + XLEN + (t - 3) * COUT
                return xins[0][:, o : o + COUT]

            def xc(lb, r0, r1):
                return (
                    xins[lb][:, WPFX + PW * r0 : WPFX + PW * r1],
                    x_d[lb][:, WPFX + PW * r0 : WPFX + PW * r1],
                )

            ring = [nc.sync, nc.scalar]
            wc0 = (xins[0][:, : WPFX + PW * CHUNKS0[0][1]],
                   x_d[0][:, : WPFX + PW * CHUNKS0[0][1]])
            w38 = (xins[0][:, WPFX + XLEN :], x_d[0][:, WPFX + XLEN :])
            c = [None] + [xc(0, r0, r1) for r0, r1 in CHUNKS0[1:]]
            i1 = [xc(1, r0, r1) for r0, r1 in CHUNKS1]
            sync_plan = [wc0, c[1], c[3], i1[1]]
            scalar_plan = [w38, c[2], c[4], i1[0], i1[2]]
            for eng, plan in ((nc.sync, sync_plan), (nc.scalar, scalar_plan)):
                for dst, src in plan:
                    eng.dma_start(dst, src)

            # --- conv: 9 accumulating matmuls per 8-row block ---
            nout = 0
            for lb in range(BPC):
                xrf = xins[lb][:, WPFX : WPFX + XLEN].rearrange("p (r c) -> p r c", c=PW)
                for yb in range(NBLK):
                    y0 = yb * ROWBLK
                    ps = pp.tile([COUT, ROWBLK * W], F32)
                    first = True
                    for dy in range(KK):
                        for dx in range(KK):
                            t = dy * KK + dx
                            nc.tensor.matmul(
                                ps[:],
                                lhs(t),
                                xrf[:, y0 + dy : y0 + dy + ROWBLK, dx : dx + W],
                                start=first,
                                stop=(dy == KK - 1 and dx == KK - 1),
                            )
                            first = False
                    if lb == BPC - 1 and yb == NBLK - 1:
                        ot = op.tile([COUT, ROWBLK * W], F32, tag="otf")
                        # final block split by PARTITION (cout halves): copies
                        # run on vector+scalar in parallel and each output DMA
                        # only needs 64 descriptors, halving the exit chain.
                        ph = COUT // 2
                        for h_, deng in ((0, nc.sync), (1, nc.scalar)):
                            sl = slice(h_ * ph, (h_ + 1) * ph)
                            if h_ == 0:
                                nc.vector.tensor_copy(ot[sl, :], ps[sl, :])
                            else:
                                nc.scalar.copy(ot[sl, :], ps[sl, :])
                            deng.dma_start(
                                o_d[lb][sl, W * y0 : W * y0 + ROWBLK * W],
                                ot[sl, :],
                            )
                    elif lb == BPC - 1 and yb == NBLK - 2:
                        # block before the final one ships alone (its pair
                        # partner takes the exit-critical path)
                        ot = op.tile([COUT, ROWBLK * W], F32, tag="ot1")
                        nc.vector.tensor_copy(ot[:], ps[:])
                        ring[nout % 2].dma_start(
                            o_d[lb][:, W * y0 : W * y0 + ROWBLK * W], ot[:]
                        )
                        nout += 1
                    else:
                        # stage two blocks per SBUF tile; ship them as one
                        # 2-block DMA (fewer DMAs = less ring pacing overhead
                        # ahead of the exit-critical final DMA)
                        if yb % 2 == 0:
                            otp = op.tile([COUT, 2 * ROWBLK * W], F32, tag="otp")
                        nc.vector.tensor_copy(
                            otp[:, (yb % 2) * ROWBLK * W : (yb % 2 + 1) * ROWBLK * W],
                            ps[:],
                        )
                        if yb % 2 == 1:
                            ring[nout % 2].dma_start(
                                o_d[lb][:, W * (y0 - ROWBLK) : W * y0 + ROWBLK * W],
                                otp[:],
                            )
                            nout += 1
    nc.compile()
    return nc


def _get_nc():
    key = ("nc_v15", WARMN)
    if key not in _CACHE:
        _CACHE[key] = _build()
    return _CACHE[key]


def kernel(x, weights):
    """x: [16,128,64,64] f32; weights: [128,128,9] f32 -> [2048,64,64] f32."""
    global LAST_EXEC_NS
    x = np.asarray(x, dtype=np.float32)
    w = np.asarray(weights, dtype=np.float32)
    # [cout, cin, k] -> [cin, k, cout] so tap k is a contiguous lhsT slice
    WPFX = 3 * COUT
    XW = WPFX + XLEN + 6 * COUT
    wT = np.ascontiguousarray(w.transpose(1, 2, 0)).reshape(CIN, KK * KK * COUT)
    wT = wT.astype(np.float16)
    xpad = np.zeros((B, CIN, XW), np.float16)
    xpad[:, :, :WPFX] = wT[:, :WPFX]
    xpad[:, :, WPFX + XLEN :] = wT[:, WPFX:]
    xpad[:, :, WPFX : WPFX + XLEN] = np.pad(
        x.astype(np.float16), ((0, 0), (0, 0), (1, 1), (1, 1))
    ).reshape(B, CIN, XLEN)

    nc = _get_nc()
    xr = xpad.reshape(NCORES, BPC, CIN, XW)
    in_maps = [{"x": np.ascontiguousarray(xr[c])} for c in range(NCORES)]

    res = bass_utils.run_bass_kernel_spmd(
        nc, in_maps, core_ids=list(range(NCORES)), trace=TRACE
    )
    LAST_EXEC_NS = res.exec_time_ns

    arr = np.stack([res.results[c]["o"] for c in range(NCORES)]).astype(np.float32)
    # out[cout*B + b] = conv[b, cout], with b = core*BPC + lb
    arr = arr.transpose(2, 0, 1, 3).reshape(COUT, B, H, W)
    return np.ascontiguousarray(arr.reshape(COUT * B, H, W))
